# revision 7
# baseline (speedup 1.0000x reference)
"""Causal multi-head attention with RoPE for Trainium2, 8-core SPMD.

Problem: B=2, S=2048, D_MODEL=1024, H=16, HD=64, causal softmax(QK^T/8)V
with interleaved-pair RoPE on q/k, projections Wq/Wk/Wv/Wo.

Sharding (host side): batch x head-group. Core c handles batch b=c//4 and
head group g=c%4 (heads 4g..4g+3, a 256-wide slice of the projection dims).
Each core computes a full [S, D_MODEL] partial of the output (its head
group's contribution through Wo); host sums 4 partials per batch.

Device layout strategy (all matmuls bf16, fp32 accumulate):
 - host passes x[b].T so the d-contraction sits on SBUF partitions
 - Q,K projected in [s, o] layout -> RoPE on DVE along free dim (pairs are
   adjacent columns) -> bf16 -> DMA-transposed (XBAR, bf16) into [o, s]
 - scores^T[k, q] = Kt.T @ Qt per 128-key block (K=64 contraction),
   accumulated into wide PSUM tiles, one Exp per wide tile (ACT),
   causal-masked by multiplying the diagonal 128x128 block; q-columns
   below the diagonal are never computed or consumed
 - PV: lhsT = [V | 1] per key block (M=65) so row 64 of the PSUM output
   accumulates the softmax denominator for free; DVE normalizes
 - o_proj consumes the attention output, PSUM is DMA'd straight to DRAM
"""

import numpy as np
import ml_dtypes

B, S, D, H = 2, 2048, 1024, 16
HD = 64
NCORES = 8
HEADS_PER_CORE = 4
GDIM = HEADS_PER_CORE * HD          # 256 projection cols per core
SB = S // 128                        # 16 s-tiles
KD = D // 128                        # 8 k-tiles over d
QCHUNK = 512
NQC = S // QCHUNK                    # 4 q-chunks
WIDE = 1536                          # wide scores psum tile (3 banks)

_BF16 = ml_dtypes.bfloat16
_cache = {}


def _build(use_rope: bool):
    import concourse.bass as bass
    import concourse.mybir as mybir
    import concourse.tile as tile
    from concourse import bacc

    F32 = mybir.dt.float32
    BF16 = mybir.dt.bfloat16
    EXP = mybir.ActivationFunctionType.Exp

    nc = bacc.Bacc(None, target_bir_lowering=False)

    xt_d = nc.dram_tensor("xt", [D, S], BF16, kind="ExternalInput")
    wqk_d = nc.dram_tensor("wqk", [D, 2 * GDIM], BF16, kind="ExternalInput")
    wv_d = nc.dram_tensor("wv", [D, GDIM], BF16, kind="ExternalInput")
    wo_d = nc.dram_tensor("wo", [GDIM, D], BF16, kind="ExternalInput")
    cos_d = nc.dram_tensor("cos8", [S, 256], BF16, kind="ExternalInput")
    sin_d = nc.dram_tensor("sin8", [S, 256], BF16, kind="ExternalInput")
    mask_d = nc.dram_tensor("maskT", [128, 128], BF16, kind="ExternalInput")
    out_d = nc.dram_tensor("out", [S, D], F32, kind="ExternalOutput")

    with tile.TileContext(nc) as tc:
        with tc.tile_pool(name="big", bufs=1) as big, \
             tc.tile_pool(name="work", bufs=3) as work, \
             tc.tile_pool(name="ropet", bufs=4) as ropet, \
             tc.tile_pool(name="pex", bufs=3) as pex:
            # ---- resident tensors ----
            xt = big.tile([128, KD, S], BF16)
            nc.sync.dma_start(xt[:], xt_d.rearrange("(k p) s -> p k s", p=128))
            wqk = big.tile([128, KD, 2 * GDIM], BF16)
            nc.sync.dma_start(wqk[:], wqk_d.rearrange("(k p) o -> p k o", p=128))
            wv = big.tile([128, KD, GDIM], BF16)
            nc.sync.dma_start(wv[:], wv_d.rearrange("(k p) o -> p k o", p=128))
            wo = big.tile([128, 2, D], BF16)
            nc.sync.dma_start(wo[:], wo_d.rearrange("(k p) o -> p k o", p=128))
            maskT = big.tile([128, 128], BF16)
            nc.sync.dma_start(maskT[:], mask_d[:])
            if use_rope:
                cos8 = big.tile([128, SB, 256], BF16)
                nc.sync.dma_start(cos8[:], cos_d.rearrange("(m p) f -> p m f", p=128))
                sin8 = big.tile([128, SB, 256], BF16)
                nc.sync.dma_start(sin8[:], sin_d.rearrange("(m p) f -> p m f", p=128))

            # attention-side resident tiles
            qkt = [big.tile([128, S], BF16, tag=f"qkt{i}", name=f"qkt{i}")
                   for i in range(4)]
            # qkt[0]: Qt heads 0-1, qkt[1]: Qt heads 2-3, qkt[2]: Kt 0-1, qkt[3]: Kt 2-3
            vsb = big.tile([128, SB, HEADS_PER_CORE * 65], BF16)
            yt2 = [big.tile([128, S], BF16, tag=f"yt2{i}", name=f"yt2{i}")
                   for i in range(2)]

            # ---- phase 1: projections + rope + transpose + V ----
            with tc.tile_pool(name="pp", bufs=4, space="PSUM") as pp:
                ones_set = False
                for m in range(SB):
                    ms = slice(m * 128, (m + 1) * 128)
                    # QK projection: [128 s, 512] = x_m @ [Wq|Wk]
                    ps = pp.tile([128, 2 * GDIM], F32, tag="ps_qk")
                    for k in range(KD):
                        nc.tensor.matmul(ps[:], xt[:, k, ms], wqk[:, k, :],
                                         start=(k == 0), stop=(k == KD - 1))
                    qkr = ropet.tile([128, 2 * GDIM], BF16, tag="qkr")
                    if use_rope:
                        pv = ps.rearrange("p (x two) -> p two x", two=2)
                        ov = qkr.rearrange("p (x two) -> p two x", two=2)
                        E, O = pv[:, 0, :], pv[:, 1, :]
                        C, Sn = cos8[:, m, :], sin8[:, m, :]
                        ta = ropet.tile([128, 256], F32, tag="ta")
                        tb = ropet.tile([128, 256], F32, tag="tb")
                        nc.vector.tensor_mul(ta[:], E, C)
                        nc.vector.tensor_mul(tb[:], O, Sn)
                        nc.vector.tensor_sub(ov[:, 0, :], ta[:], tb[:])
                        tc_ = ropet.tile([128, 256], F32, tag="tc")
                        td = ropet.tile([128, 256], F32, tag="td")
                        nc.vector.tensor_mul(tc_[:], O, C)
                        nc.vector.tensor_mul(td[:], E, Sn)
                        nc.vector.tensor_add(ov[:, 1, :], tc_[:], td[:])
                    else:
                        nc.vector.tensor_copy(qkr[:], ps[:])
                    # transpose 128x128 blocks into qkt tiles
                    for cb in range(4):
                        nc.sync.dma_start_transpose(
                            qkt[cb][:, ms], qkr[:, cb * 128:(cb + 1) * 128])

                    # V projection: [128 s, 256]
                    psv = pp.tile([128, GDIM], F32, tag="ps_v")
                    for k in range(KD):
                        nc.tensor.matmul(psv[:], xt[:, k, ms], wv[:, k, :],
                                         start=(k == 0), stop=(k == KD - 1))
                    if not ones_set:
                        nc.vector.memset(vsb[:], 1.0)
                        ones_set = True
                    # copy 4 head blocks of 64 into stride-65 slots
                    dst = vsb[:, m, :].rearrange("p (h c) -> p h c", h=4)[:, :, 0:64]
                    src = psv.rearrange("p (h c) -> p h c", h=4)
                    nc.vector.tensor_copy(dst, src)

            # ---- phase 2: attention per head ----
            with tc.tile_pool(name="sc", bufs=2, space="PSUM") as scp, \
                 tc.tile_pool(name="yt", bufs=2, space="PSUM") as ytp:
                for h in range(HEADS_PER_CORE):
                    qt = qkt[h // 2]
                    kt = qkt[2 + h // 2]
                    r0 = (h % 2) * 64
                    rows = slice(r0, r0 + 64)
                    vcol = slice(h * 65, h * 65 + 65)
                    for qc in range(NQC):
                        q0 = qc * QCHUNK
                        # Pack kb blocks into wide psum tiles of WIDE cols
                        # (3 banks). A matmul may not cross a 512-col psum
                        # bank, so emit widths in order 512...512,384,128,256
                        # which tiles banks exactly (384+128=512).
                        order = list(range(4 * qc)) + \
                            [4 * qc, 4 * qc + 1, 4 * qc + 3, 4 * qc + 2]
                        groups, cur = [], []
                        cols = 0
                        for kb in order:
                            r = max(0, kb - 4 * qc)
                            qoff, n = q0 + r * 128, QCHUNK - r * 128
                            if cols + n > WIDE:
                                groups.append(cur)
                                cur, cols = [], 0
                            cur.append((kb, qoff, n, cols))
                            cols += n
                        groups.append(cur)

                        ytps = ytp.tile([65, QCHUNK], F32, tag="ytps")
                        last_kb = groups[-1][-1][0]
                        for grp in groups:
                            gcols = grp[-1][3] + grp[-1][2]
                            sc = scp.tile([128, WIDE], F32, tag="sc")
                            for (kb, qoff, n, o) in grp:
                                nc.tensor.matmul(
                                    sc[:, o:o + n],
                                    kt[rows, kb * 128:(kb + 1) * 128],
                                    qt[rows, qoff:qoff + n],
                                    start=True, stop=True)
                            pe = pex.tile([128, WIDE], BF16, tag="pe")
                            nc.scalar.activation(pe[:, :gcols], sc[:, :gcols],
                                                 EXP, scale=0.125)
                            for (kb, qoff, n, o) in grp:
                                if kb >= 4 * qc:  # diagonal block: causal mask
                                    nc.vector.tensor_mul(
                                        pe[:, o:o + 128], pe[:, o:o + 128],
                                        maskT[:])
                                # kb==0 always has n=512: start clears the
                                # whole [65, QCHUNK] accumulator region
                                nc.tensor.matmul(
                                    ytps[:, qoff - q0:qoff - q0 + n],
                                    vsb[:, kb, vcol],
                                    pe[:, o:o + n],
                                    start=(kb == 0), stop=(kb == last_kb))
                        rc = work.tile([1, QCHUNK], F32, tag="rc")
                        nc.vector.reciprocal(rc[:], ytps[64:65, :])
                        bc = work.tile([64, QCHUNK], F32, tag="bc")
                        nc.gpsimd.partition_broadcast(bc[:], rc[0:1, :])
                        nc.vector.tensor_mul(
                            yt2[h // 2][r0:r0 + 64, q0:q0 + QCHUNK],
                            ytps[0:64, :], bc[:])

            # ---- phase 3: o_proj ----
            with tc.tile_pool(name="op", bufs=4, space="PSUM") as op:
                for m in range(SB):
                    ms = slice(m * 128, (m + 1) * 128)
                    for nb in range(2):
                        po = op.tile([128, 512], F32, tag="po")
                        for k in range(2):
                            nc.tensor.matmul(po[:], yt2[k][:, ms],
                                             wo[:, k, nb * 512:(nb + 1) * 512],
                                             start=(k == 0), stop=(k == 1))
                        so = work.tile([128, 512], F32, tag="so")
                        nc.any.tensor_copy(so[:], po[:])
                        nc.sync.dma_start(
                            out_d[ms, nb * 512:(nb + 1) * 512], so[:])
    nc.compile()
    return nc


def _prep_core_inputs(x, Wq, Wk, Wv, Wo, cos_g, sin_g, use_rope):
    """Host-side shard + layout prep. Returns list of 8 input dicts."""
    maskT = np.tril(np.ones((128, 128), np.float32)).T.astype(_BF16)
    # interleave cos/sin to the 256-wide repeating pattern used by rope
    cos8 = np.tile(cos_g, (1, 8)).astype(_BF16)
    sin8 = np.tile(sin_g, (1, 8)).astype(_BF16)
    maps = []
    for c in range(NCORES):
        b, g = divmod(c, HEADS_PER_CORE)
        rows = slice(g * GDIM, (g + 1) * GDIM)
        wqk = np.concatenate([Wq[rows], Wk[rows]], axis=0).T  # [D, 512]
        maps.append({
            "xt": np.ascontiguousarray(x[b].T).astype(_BF16),
            "wqk": np.ascontiguousarray(wqk).astype(_BF16),
            "wv": np.ascontiguousarray(Wv[rows].T).astype(_BF16),
            "wo": np.ascontiguousarray(Wo[:, rows].T).astype(_BF16),
            "cos8": cos8,
            "sin8": sin8,
            "maskT": maskT,
        })
    return maps


def kernel(x, token_positions, use_rope, Wq, Wk, Wv, Wo, cos, sin):
    from concourse.bass_utils import run_bass_kernel_spmd

    x = np.asarray(x, np.float32)
    token_positions = np.asarray(token_positions)
    Wq = np.asarray(Wq, np.float32)
    Wk = np.asarray(Wk, np.float32)
    Wv = np.asarray(Wv, np.float32)
    Wo = np.asarray(Wo, np.float32)
    cos = np.asarray(cos, np.float32)
    sin = np.asarray(sin, np.float32)
    rope = bool(int(use_rope))

    cos_g = cos[token_positions]  # [S, 32]
    sin_g = sin[token_positions]

    if rope not in _cache:
        _cache[rope] = _build(rope)
    nc = _cache[rope]

    in_maps = _prep_core_inputs(x, Wq, Wk, Wv, Wo, cos_g, sin_g, rope)
    res = run_bass_kernel_spmd(nc, in_maps, list(range(NCORES)))

    out = np.zeros((B, S, D), np.float32)
    for c in range(NCORES):
        out[c // HEADS_PER_CORE] += res.results[c]["out"]
    return out
